# revision 32
# baseline (speedup 1.0000x reference)
"""Trainium2 Bass kernel for batched 9x9-token MHSA with decomposed relative
position bias (1x1-conv QKV projection).

Strategy: pure data parallel over batch (B=1024 -> 128 per core x 8 cores).
All matmuls run on 16-bit operands (fp16 GEMM/scores, bf16 AV) — fp32/fp32r
matmuls measure ~2.5-3.5 cycles/column on TRN2 HW while 16-bit runs at 1.

Per core:
  - QK projection GEMM channel-major (out [o, (b,n)]), fp16 operands,
    N=324 moving columns. Relative-position table R = rel_h+rel_w (+ k bias)
    is folded into K during the PSUM->SBUF epilogue, so scores = Q.(K+R).
  - V projection GEMM token-major over 128-row windows that span batch
    boundaries (full PE column utilization), fp16, N=512 moving; batch-
    aligned v tiles assembled via contiguous SBUF->SBUF shift DMAs.
  - Scores computed transposed: S^T[m,n] = sum_d k'[d,m] q[d,n] via
    matmul(lhsT=k', rhs=q) in fp16. Softmax runs over partitions (m): no max
    subtraction (logits bounded by ~33, exp<=1.4e14, safe in fp32);
    denominator obtained by appending a ones-row to V so the AV matmul
    emits unnormalized output rows 0..63 and the denominator in row 64.
  - exp on ScalarE (fp32 PSUM -> bf16 SBUF), AV matmul in bf16.
  - Output stored bf16 (halves write traffic); the final normalize division
    happens on the host during unsharding.

Self-contained: hardcodes B=1024, DM=512, H=8, D=64, N=81, 8 cores.
"""

import os
import sys

import numpy as np

for _p in ("/opt/trn_rl_repo", "/root/.axon_site/_ro/trn_rl_repo"):
    if os.path.isdir(_p) and _p not in sys.path:
        sys.path.insert(0, _p)

import concourse.bass as bass
import concourse.tile as tile
from concourse import bacc
from concourse import mybir
from concourse.alu_op_type import AluOpType
from concourse.bass_utils import run_bass_kernel_spmd

F32 = mybir.dt.float32
F16 = mybir.dt.float16
BF16 = mybir.dt.bfloat16
AF = mybir.ActivationFunctionType

B, DM, H, D, N = 1024, 512, 8, 64, 81
NCORES = 8
B_CORE = B // NCORES  # 128
NB = 4                # batches per chunk
NCOLS = NB * N        # 324 GEMM moving columns per chunk


def build_kernel(n_b=B_CORE):
    assert n_b % NB == 0
    nchunks = n_b // NB

    nc = bacc.Bacc()
    # x pre-transposed on host to channel-major [DM, n_b*N] so every DMA row
    # is a contiguous run; fp16 halves the HBM read traffic.
    xd = nc.dram_tensor("x", [DM, n_b * N], F16, kind="ExternalInput")
    wtd = nc.dram_tensor("wt", [DM, 3 * DM], F16, kind="ExternalInput")  # W^T
    bqd = nc.dram_tensor("bq", [DM, 1], F32, kind="ExternalInput")       # q bias
    rpd = nc.dram_tensor("rp", [DM, N], F32, kind="ExternalInput")       # rel_h+rel_w+bk
    # out in device-native layout [pair][d+denom][b par hh n]; row D holds the
    # softmax denominator — the final normalize division happens on the host
    # during unsharding. One fully-contiguous store per batch pair.
    outd = nc.dram_tensor(
        "out", [n_b // 2, D + 1, 2 * 2 * 4 * N], BF16, kind="ExternalOutput"
    )

    with tile.TileContext(nc) as tc:
        with (
            tc.tile_pool(name="const", bufs=1) as cpool,
            tc.tile_pool(name="xin", bufs=3) as xpool,
            tc.tile_pool(name="qk", bufs=2) as qkpool,
            tc.tile_pool(name="vaug", bufs=8) as vpool,
            tc.tile_pool(name="vstage", bufs=6) as svpool,
            tc.tile_pool(name="emat", bufs=4) as epool,
            tc.tile_pool(name="outs", bufs=3) as opool,
            tc.tile_pool(name="ps_qk", bufs=3, space="PSUM") as ps_qk,
            tc.tile_pool(name="ps_v", bufs=1, space="PSUM") as ps_v,
            tc.tile_pool(name="ps_s", bufs=2, space="PSUM") as ps_s,
            tc.tile_pool(name="ps_av", bufs=2, space="PSUM") as ps_av,
        ):
            # ---- first x chunk + constants; ordered so the first QK matmul
            # group unblocks as early as possible: x(0) and the q-section of
            # the weights land first, then k/v sections and the small tables.
            def load_x(c):
                b0 = c * NB
                xt = []
                for kc in range(4):
                    t = xpool.tile([128, NB, N], F16, tag=f"x{kc}")
                    nc.sync.dma_start(
                        out=t[:].rearrange("p b n -> p (b n)"),
                        in_=xd[kc * 128:(kc + 1) * 128, b0 * N:(b0 + NB) * N],
                    )
                    xt.append(t)
                return xt

            # warm-up: the PE clock gate (HAM) starts at half rate and needs
            # ~3.4us of sustained activity to release. The first ~12us of the
            # kernel are DMA wait anyway, so burn them on dummy matmuls over
            # never-initialized scratch (results discarded) to enter the real
            # GEMM at full clock.
            scratch = cpool.tile([128, NCOLS], F16, tag="scratch")
            nc.vector.memset(scratch[:], 0.0)
            wps = ps_qk.tile([128, NCOLS], F32, tag="psqk", name="warm")
            for _ in range(60):
                nc.tensor.matmul(
                    wps[:], lhsT=scratch[:, 0:128], rhs=scratch[:],
                    start=True, stop=True,
                )

            xt0 = load_x(0)
            wt = [
                cpool.tile([128, 3 * DM], F16, tag=f"wt{kc}", name=f"wt{kc}")
                for kc in range(4)
            ]
            for sec in range(3):  # q, k, v weight column sections
                for kc in range(4):
                    nc.sync.dma_start(
                        out=wt[kc][:, sec * DM:(sec + 1) * DM],
                        in_=wtd[kc * 128:(kc + 1) * 128, sec * DM:(sec + 1) * DM],
                    )
                if sec == 0:
                    rp = []
                    for mo in range(4):
                        t = cpool.tile([128, N], F32, tag=f"rp{mo}")
                        nc.sync.dma_start(
                            out=t[:], in_=rpd[mo * 128:(mo + 1) * 128, :]
                        )
                        rp.append(t)
                elif sec == 1:
                    bq = []
                    for mo in range(4):
                        t = cpool.tile([128, 1], F32, tag=f"bq{mo}")
                        nc.sync.dma_start(
                            out=t[:], in_=bqd[mo * 128:(mo + 1) * 128, :]
                        )
                        bq.append(t)
            state = {}  # carries one chunk's tiles to the next iteration

            # V window segment map: chunk token-row g = 81*j + t is computed
            # in window w at row g-WB[w] and must land in va[j] row t.
            # Uneven boundaries [0,128,243,324) make batches 0 AND 3 start
            # window-aligned so both alias their window tile (no shift DMA).
            # (w, src_row0, src_row1, batch j, dst_row0)
            WB = [0, 128, 243, NB * N]
            vsegs = []
            for j in range(NB):
                t = 0
                while t < N:
                    g = j * N + t
                    w = max(i for i in range(3) if WB[i] <= g)
                    seg = min(N - t, WB[w + 1] - g)
                    vsegs.append((w, g - WB[w], g - WB[w] + seg, j, t))
                    t += seg
            aliased = {
                j: segs[0][0]
                for j in range(NB)
                if len(segs := [s for s in vsegs if s[3] == j]) == 1
                and segs[0][1] == 0
            }

            def gemm(c, xt=None):
                b0 = c * NB
                if xt is None:
                    xt = load_x(c)

                # q,k channel-major GEMM: out[o, (b,n)] for o in 0..1024
                q_sb, k_sb = [], []
                for mo in range(8):
                    ps = ps_qk.tile([128, NCOLS], F32, tag="psqk")
                    for kc in range(4):
                        nc.tensor.matmul(
                            ps[:],
                            lhsT=wt[kc][:, mo * 128:(mo + 1) * 128],
                            rhs=xt[kc][:].rearrange("p b n -> p (b n)"),
                            start=(kc == 0),
                            stop=(kc == 3),
                        )
                    if mo < 4:  # q: add bias on ScalarE while copying out
                        t = qkpool.tile([128, NCOLS], F16, tag=f"q{mo}")
                        nc.scalar.activation(t[:], ps[:], AF.Identity, bias=bq[mo][:])
                        q_sb.append(t)
                    else:  # k: add (rel bias + k bias), broadcast over batch
                        # tile padded to NCOLS+47 so every scores lhsT slice
                        # can be 128 columns wide -> LDWEIGHTS qualifies for
                        # fast weight load. Pad contents are never-read junk
                        # (scores output rows 81..127 are ignored).
                        t = qkpool.tile([128, NCOLS + 47], F16, tag=f"k{mo - 4}")
                        nc.vector.tensor_tensor(
                            t[:, 0:NCOLS].rearrange("p (b n) -> p b n", b=NB),
                            ps[:].rearrange("p (b n) -> p b n", b=NB),
                            rp[mo - 4][:].unsqueeze(1).broadcast_to([128, NB, N]),
                            AluOpType.add,
                        )
                        k_sb.append(t)

                # v token-major GEMM in 128-row windows spanning batch
                # boundaries (full PE column utilization: 3 windows/chunk
                # instead of 4 81-row batches). No bias — host adds it after
                # the final normalize. Misaligned rows reach their va tile by
                # SBUF->SBUF shift DMA via a bf16 staging tile.
                # staging is written in the augmented (h, d|1) layout by the
                # DVE cast (strided dst is free: PSUM src caps DVE at 1x
                # anyway), so every shift DMA is fully contiguous rows.
                sv = []
                for w in range(3):
                    r0, r1 = WB[w], WB[w + 1]
                    rows = r1 - r0
                    ps = ps_v.tile([128, DM], F32, tag="psv")
                    for kc in range(4):
                        nc.tensor.matmul(
                            ps[0:rows, :],
                            lhsT=xt[kc][:].rearrange("p b n -> p (b n)")[:, r0:r1],
                            rhs=wt[kc][:, 2 * DM:3 * DM],
                            start=(kc == 0),
                            stop=(kc == 3),
                        )
                    t = svpool.tile([128, H * (D + 1)], BF16, tag="sv", name=f"sv{w}")
                    nc.vector.tensor_copy(
                        t[0:rows].rearrange("p (h e) -> p h e", h=H)[:, :, 0:D],
                        ps[0:rows, :].rearrange("p (h d) -> p h d", h=H),
                    )
                    nc.vector.memset(
                        t[0:rows].rearrange("p (h e) -> p h e", h=H)[:, :, D:D + 1],
                        1.0,
                    )
                    sv.append(t)
                # window-aligned batches alias their window tile directly;
                # the rest are assembled by contiguous shift DMAs.
                v_aug = []
                for j in range(NB):
                    if j in aliased:
                        v_aug.append(sv[aliased[j]])
                    else:
                        va = vpool.tile(
                            [N, H * (D + 1)], BF16, tag="vaug", name=f"va{j}"
                        )
                        v_aug.append(va)
                for w, s0, s1, j, t0 in vsegs:
                    if j in aliased:
                        continue
                    nc.sync.dma_start(
                        out=v_aug[j][t0:t0 + (s1 - s0), :],
                        in_=sv[w][s0:s1, :],
                    )
                return {"q": q_sb, "k": k_sb, "v": v_aug, "b0": b0}

            def attention(st):
                q_sb, k_sb, v_aug, b0 = st["q"], st["k"], st["v"], st["b0"]
                ot = None
                for j in range(NB):
                    b = b0 + j
                    js = slice(j * N, (j + 1) * N)
                    if j % 2 == 0:  # one output tile per batch pair
                        ot = opool.tile([D + 1, 2 * 2 * 4 * N], BF16, tag="ot")
                    # scores transposed: S^T = k'.T-contracted over d.
                    # Grouped by head parity: tile `par` holds heads 2*hh+par,
                    # so every matmul into one PSUM tile has the same lhsT
                    # base partition (mixing 0/64 in one fp32 group breaks HW).
                    # parities interleaved: consecutive matmuls use disjoint
                    # PE row strips (0-63 vs 64-127) and different PSUM banks,
                    # so the PE can overlap them
                    psS = [
                        ps_s.tile([128, 4 * N], F32, tag="pss", name=f"pss{j}_{p}")
                        for p in range(2)
                    ]
                    # par-outer: all of par0's scores issue first so its exp
                    # (the AV gate) starts a few slots earlier
                    for par in range(2):
                        for hh in range(4):
                            po = par * 64
                            # 128-wide lhsT slice (81 real tokens + padding)
                            # enables FWL on the weight load; output rows
                            # 81..127 are junk and never read.
                            nc.tensor.matmul(
                                psS[par][:, hh * N:(hh + 1) * N],
                                lhsT=k_sb[hh][po:po + 64, j * N:j * N + 128],
                                rhs=q_sb[hh][po:po + 64, js],
                                start=True,
                                stop=True,
                                tile_position=(po, 0),
                            )
                    emat = []
                    for par in range(2):
                        e = epool.tile([N, 4 * N], BF16, tag="e")
                        nc.scalar.activation(e[:], psS[par][0:N, :], AF.Exp)
                        emat.append(e)
                    # AV with ones-row: rows 0..63 unnormalized out, row 64 denom
                    psA = []
                    for par in range(2):
                        ps = ps_av.tile([D + 1, 4 * N], F32, tag="psav")
                        for hh in range(4):
                            h = 2 * hh + par
                            nc.tensor.matmul(
                                ps[:, hh * N:(hh + 1) * N],
                                lhsT=v_aug[j][0:N, h * (D + 1):(h + 1) * (D + 1)],
                                rhs=emat[par][:, hh * N:(hh + 1) * N],
                                start=True,
                                stop=True,
                            )
                        psA.append(ps)
                    # ot free layout is (b01, par, hh, n); channel h = 2*hh+par
                    # one copy on DVE, one on ACT to balance engine load
                    joff = (j % 2) * 2 * 4 * N
                    nc.vector.tensor_copy(
                        ot[:, joff:joff + 4 * N], psA[0][:]
                    )
                    nc.scalar.activation(
                        ot[:, joff + 4 * N:joff + 2 * 4 * N], psA[1][:], AF.Identity
                    )
                    if j % 2 == 1:
                        nc.sync.dma_start(out=outd[b // 2], in_=ot[:])

            # software pipeline: attention for chunk c-1 is emitted before
            # GEMM for chunk c so PE never stalls on ACT/DVE epilogues
            for c in range(nchunks + 1):
                if c > 0:
                    attention(state)
                if c < nchunks:
                    state = gemm(c, xt=xt0 if c == 0 else None)

    if not nc.is_finalized():
        nc.finalize()
    return nc


_CACHE = {}


def _get_nc(n_b):
    if n_b not in _CACHE:
        _CACHE[n_b] = build_kernel(n_b)
    return _CACHE[n_b]


def _prep_inputs(x, qkv_w, qkv_b, rel_h, rel_w):
    # per-core channel-major x: [NCORES][DM, B_CORE*N] in fp16
    x = np.asarray(x, dtype=np.float32).reshape(B, DM, N)
    x = np.ascontiguousarray(
        x.reshape(NCORES, B_CORE, DM, N).transpose(0, 2, 1, 3)
    ).reshape(NCORES, DM, B_CORE * N).astype(np.float16)
    qkv_w = np.asarray(qkv_w, dtype=np.float32)
    qkv_b = np.asarray(qkv_b, dtype=np.float32)
    wt = np.ascontiguousarray(qkv_w.T).astype(np.float16)               # [512, 1536]
    bq = np.ascontiguousarray(qkv_b[0:DM].reshape(DM, 1))
    rel = (np.asarray(rel_h, np.float32) + np.asarray(rel_w, np.float32))
    rp = np.ascontiguousarray(rel.reshape(DM, N) + qkv_b[DM:2 * DM].reshape(DM, 1))
    bv = np.ascontiguousarray(qkv_b[2 * DM:3 * DM])
    return x, wt, bq, rp, bv


def kernel(x, qkv_w, qkv_b, rel_h, rel_w, _trace=False):
    xs, wt, bq, rp, bv = _prep_inputs(x, qkv_w, qkv_b, rel_h, rel_w)
    nc = _get_nc(B_CORE)
    in_maps = [
        {"x": xs[i], "wt": wt, "bq": bq, "rp": rp}
        for i in range(NCORES)
    ]
    res = run_bass_kernel_spmd(
        nc, in_maps, core_ids=list(range(NCORES)), trace=_trace
    )
    # decode device layout [pair, d|denom, b01, par, hh, n] -> [B, DM, N];
    # row D is the softmax denominator (normalize here during unshard), and
    # the v bias (linear in the attention average) is added after the divide
    out = np.stack(
        [np.asarray(r["out"]).astype(np.float32) for r in res.results], axis=0
    )
    out = out.reshape(NCORES, B_CORE // 2, D + 1, 2, 2, 4, N)
    out = out[:, :, 0:D] / out[:, :, D:D + 1]
    # axes now [core, pair, d, b01, par, hh, n]; channel h = 2*hh + par
    out = out + bv.reshape(4, 2, D).transpose(2, 1, 0)[None, None, :, None, :, :, None]
    out = out.transpose(0, 1, 3, 5, 4, 2, 6)  # core, pair, b01, hh, par, d, n
    out = out.reshape(B, DM, N)
    if _trace:
        kernel.last_results = res
    return np.ascontiguousarray(out.reshape(B, DM, 9, 9))


# revision 34
# speedup vs baseline: 1.0141x; 1.0141x over previous
"""Trainium2 Bass kernel for batched 9x9-token MHSA with decomposed relative
position bias (1x1-conv QKV projection).

Strategy: pure data parallel over batch (B=1024 -> 128 per core x 8 cores).
All matmuls run on 16-bit operands (fp16 GEMM/scores, bf16 AV) — fp32/fp32r
matmuls measure ~2.5-3.5 cycles/column on TRN2 HW while 16-bit runs at 1.

Per core:
  - QK projection GEMM channel-major (out [o, (b,n)]), fp16 operands,
    N=324 moving columns. Relative-position table R = rel_h+rel_w (+ k bias)
    is folded into K during the PSUM->SBUF epilogue, so scores = Q.(K+R).
  - V projection GEMM token-major over 128-row windows that span batch
    boundaries (full PE column utilization), fp16, N=512 moving; batch-
    aligned v tiles assembled via contiguous SBUF->SBUF shift DMAs.
  - Scores computed transposed: S^T[m,n] = sum_d k'[d,m] q[d,n] via
    matmul(lhsT=k', rhs=q) in fp16. Softmax runs over partitions (m): no max
    subtraction (logits bounded by ~33, exp<=1.4e14, safe in fp32);
    denominator obtained by appending a ones-row to V so the AV matmul
    emits unnormalized output rows 0..63 and the denominator in row 64.
  - exp on ScalarE (fp32 PSUM -> bf16 SBUF), AV matmul in bf16.
  - Output stored bf16 (halves write traffic); the final normalize division
    happens on the host during unsharding.

Self-contained: hardcodes B=1024, DM=512, H=8, D=64, N=81, 8 cores.
"""

import os
import sys

import numpy as np

for _p in ("/opt/trn_rl_repo", "/root/.axon_site/_ro/trn_rl_repo"):
    if os.path.isdir(_p) and _p not in sys.path:
        sys.path.insert(0, _p)

import concourse.bass as bass
import concourse.tile as tile
from concourse import bacc
from concourse import mybir
from concourse.alu_op_type import AluOpType
from concourse.bass_utils import run_bass_kernel_spmd

F32 = mybir.dt.float32
F16 = mybir.dt.float16
BF16 = mybir.dt.bfloat16
AF = mybir.ActivationFunctionType

B, DM, H, D, N = 1024, 512, 8, 64, 81
NCORES = 8
B_CORE = B // NCORES  # 128
NB = 4                # batches per chunk
NCOLS = NB * N        # 324 GEMM moving columns per chunk


def build_kernel(n_b=B_CORE):
    assert n_b % NB == 0
    nchunks = n_b // NB

    nc = bacc.Bacc()
    # x pre-transposed on host to channel-major [DM, n_b*N] so every DMA row
    # is a contiguous run; fp16 halves the HBM read traffic.
    xd = nc.dram_tensor("x", [DM, n_b * N], F16, kind="ExternalInput")
    wtd = nc.dram_tensor("wt", [DM, 3 * DM], F16, kind="ExternalInput")  # W^T
    bqd = nc.dram_tensor("bq", [DM, 1], F32, kind="ExternalInput")       # q bias
    rpd = nc.dram_tensor("rp", [DM, N], F32, kind="ExternalInput")       # rel_h+rel_w+bk
    # out in device-native layout [pair][d+denom][b par hh n]; row D holds the
    # softmax denominator — the final normalize division happens on the host
    # during unsharding. One fully-contiguous store per batch pair.
    outd = nc.dram_tensor(
        "out", [n_b // 2, D + 1, 2 * 2 * 4 * N], BF16, kind="ExternalOutput"
    )

    with tile.TileContext(nc) as tc:
        with (
            tc.tile_pool(name="const", bufs=1) as cpool,
            tc.tile_pool(name="xin", bufs=3) as xpool,
            tc.tile_pool(name="qk", bufs=2) as qkpool,
            tc.tile_pool(name="vaug", bufs=8) as vpool,
            tc.tile_pool(name="vstage", bufs=6) as svpool,
            tc.tile_pool(name="emat", bufs=4) as epool,
            tc.tile_pool(name="outs", bufs=3) as opool,
            tc.tile_pool(name="ps_qk", bufs=3, space="PSUM") as ps_qk,
            tc.tile_pool(name="ps_v", bufs=1, space="PSUM") as ps_v,
            tc.tile_pool(name="ps_s", bufs=2, space="PSUM") as ps_s,
            tc.tile_pool(name="ps_av", bufs=2, space="PSUM") as ps_av,
        ):
            # ---- first x chunk + constants; ordered so the first QK matmul
            # group unblocks as early as possible: x(0) and the q-section of
            # the weights land first, then k/v sections and the small tables.
            def load_x(c):
                b0 = c * NB
                xt = []
                for kc in range(4):
                    t = xpool.tile([128, NB, N], F16, tag=f"x{kc}")
                    nc.sync.dma_start(
                        out=t[:].rearrange("p b n -> p (b n)"),
                        in_=xd[kc * 128:(kc + 1) * 128, b0 * N:(b0 + NB) * N],
                    )
                    xt.append(t)
                return xt

            # warm-up: the PE clock gate (HAM) starts at half rate and needs
            # ~3.4us of sustained activity to release. The first ~12us of the
            # kernel are DMA wait anyway, so burn them on dummy matmuls over
            # never-initialized scratch (results discarded) to enter the real
            # GEMM at full clock.
            scratch = cpool.tile([128, NCOLS], F16, tag="scratch")
            nc.vector.memset(scratch[:], 0.0)
            wps = ps_qk.tile([128, NCOLS], F32, tag="psqk", name="warm")
            for _ in range(60):
                nc.tensor.matmul(
                    wps[:], lhsT=scratch[:, 0:128], rhs=scratch[:],
                    start=True, stop=True,
                )

            xt0 = load_x(0)
            wt = [
                cpool.tile([128, 3 * DM], F16, tag=f"wt{kc}", name=f"wt{kc}")
                for kc in range(4)
            ]
            for sec in range(3):  # q, k, v weight column sections
                for kc in range(4):
                    nc.sync.dma_start(
                        out=wt[kc][:, sec * DM:(sec + 1) * DM],
                        in_=wtd[kc * 128:(kc + 1) * 128, sec * DM:(sec + 1) * DM],
                    )
                if sec == 0:
                    rp = []
                    for mo in range(4):
                        t = cpool.tile([128, N], F32, tag=f"rp{mo}")
                        nc.sync.dma_start(
                            out=t[:], in_=rpd[mo * 128:(mo + 1) * 128, :]
                        )
                        rp.append(t)
                elif sec == 1:
                    bq = []
                    for mo in range(4):
                        t = cpool.tile([128, 1], F32, tag=f"bq{mo}")
                        nc.sync.dma_start(
                            out=t[:], in_=bqd[mo * 128:(mo + 1) * 128, :]
                        )
                        bq.append(t)
            state = {}  # carries one chunk's tiles to the next iteration

            # V window segment map: chunk token-row g = 81*j + t is computed
            # in window w at row g-WB[w] and must land in va[j] row t.
            # Uneven boundaries [0,128,243,324) make batches 0 AND 3 start
            # window-aligned so both alias their window tile (no shift DMA).
            # (w, src_row0, src_row1, batch j, dst_row0)
            WB = [0, 128, 243, NB * N]
            vsegs = []
            for j in range(NB):
                t = 0
                while t < N:
                    g = j * N + t
                    w = max(i for i in range(3) if WB[i] <= g)
                    seg = min(N - t, WB[w + 1] - g)
                    vsegs.append((w, g - WB[w], g - WB[w] + seg, j, t))
                    t += seg
            aliased = {
                j: segs[0][0]
                for j in range(NB)
                if len(segs := [s for s in vsegs if s[3] == j]) == 1
                and segs[0][1] == 0
            }

            def gemm(c, xt=None):
                b0 = c * NB
                if xt is None:
                    xt = load_x(c)

                # q,k channel-major GEMM: out[o, (b,n)] for o in 0..1024
                q_sb, k_sb = [], []
                for mo in range(8):
                    ps = ps_qk.tile([128, NCOLS], F32, tag="psqk")
                    for kc in range(4):
                        nc.tensor.matmul(
                            ps[:],
                            lhsT=wt[kc][:, mo * 128:(mo + 1) * 128],
                            rhs=xt[kc][:].rearrange("p b n -> p (b n)"),
                            start=(kc == 0),
                            stop=(kc == 3),
                        )
                    if mo < 4:  # q: add bias on ScalarE while copying out
                        t = qkpool.tile([128, NCOLS], F16, tag=f"q{mo}")
                        nc.scalar.activation(t[:], ps[:], AF.Identity, bias=bq[mo][:])
                        q_sb.append(t)
                    else:  # k: add (rel bias + k bias), broadcast over batch
                        # tile padded to NCOLS+47 so every scores lhsT slice
                        # can be 128 columns wide -> LDWEIGHTS qualifies for
                        # fast weight load. Pad contents are never-read junk
                        # (scores output rows 81..127 are ignored).
                        t = qkpool.tile([128, NCOLS + 47], F16, tag=f"k{mo - 4}")
                        nc.vector.tensor_tensor(
                            t[:, 0:NCOLS].rearrange("p (b n) -> p b n", b=NB),
                            ps[:].rearrange("p (b n) -> p b n", b=NB),
                            rp[mo - 4][:].unsqueeze(1).broadcast_to([128, NB, N]),
                            AluOpType.add,
                        )
                        k_sb.append(t)

                # v token-major GEMM in 128-row windows spanning batch
                # boundaries (full PE column utilization: 3 windows/chunk
                # instead of 4 81-row batches). No bias — host adds it after
                # the final normalize. Misaligned rows reach their va tile by
                # SBUF->SBUF shift DMA via a bf16 staging tile.
                # staging is written in the augmented (h, d|1) layout by the
                # DVE cast (strided dst is free: PSUM src caps DVE at 1x
                # anyway), so every shift DMA is fully contiguous rows.
                sv = []
                for w in range(3):
                    r0, r1 = WB[w], WB[w + 1]
                    rows = r1 - r0
                    ps = ps_v.tile([128, DM], F32, tag="psv")
                    for kc in range(4):
                        nc.tensor.matmul(
                            ps[0:rows, :],
                            lhsT=xt[kc][:].rearrange("p b n -> p (b n)")[:, r0:r1],
                            rhs=wt[kc][:, 2 * DM:3 * DM],
                            start=(kc == 0),
                            stop=(kc == 3),
                        )
                    t = svpool.tile([128, H * (D + 1)], BF16, tag="sv", name=f"sv{w}")
                    nc.vector.tensor_copy(
                        t[0:rows].rearrange("p (h e) -> p h e", h=H)[:, :, 0:D],
                        ps[0:rows, :].rearrange("p (h d) -> p h d", h=H),
                    )
                    nc.vector.memset(
                        t[0:rows].rearrange("p (h e) -> p h e", h=H)[:, :, D:D + 1],
                        1.0,
                    )
                    sv.append(t)
                # window-aligned batches alias their window tile directly;
                # the rest are assembled by contiguous shift DMAs.
                v_aug = []
                for j in range(NB):
                    if j in aliased:
                        v_aug.append(sv[aliased[j]])
                    else:
                        va = vpool.tile(
                            [N, H * (D + 1)], BF16, tag="vaug", name=f"va{j}"
                        )
                        v_aug.append(va)
                for w, s0, s1, j, t0 in vsegs:
                    if j in aliased:
                        continue
                    nc.sync.dma_start(
                        out=v_aug[j][t0:t0 + (s1 - s0), :],
                        in_=sv[w][s0:s1, :],
                    )
                return {"q": q_sb, "k": k_sb, "v": v_aug, "b0": b0}

            def attention(st):
                q_sb, k_sb, v_aug, b0 = st["q"], st["k"], st["v"], st["b0"]
                ot = None
                for j in range(NB):
                    b = b0 + j
                    js = slice(j * N, (j + 1) * N)
                    if j % 2 == 0:  # one output tile per batch pair
                        ot = opool.tile([D + 1, 2 * 2 * 4 * N], BF16, tag="ot")
                    # scores transposed: S^T = k'.T-contracted over d.
                    # Grouped by head parity: tile `par` holds heads 2*hh+par,
                    # so every matmul into one PSUM tile has the same lhsT
                    # base partition (mixing 0/64 in one fp32 group breaks HW).
                    # parities interleaved: consecutive matmuls use disjoint
                    # PE row strips (0-63 vs 64-127) and different PSUM banks,
                    # so the PE can overlap them
                    psS = [
                        ps_s.tile([128, 4 * N], F32, tag="pss", name=f"pss{j}_{p}")
                        for p in range(2)
                    ]
                    for hh in range(4):
                        for par in range(2):
                            po = par * 64
                            # 128-wide lhsT slice (81 real tokens + padding)
                            # enables FWL on the weight load; output rows
                            # 81..127 are junk and never read.
                            nc.tensor.matmul(
                                psS[par][:, hh * N:(hh + 1) * N],
                                lhsT=k_sb[hh][po:po + 64, j * N:j * N + 128],
                                rhs=q_sb[hh][po:po + 64, js],
                                start=True,
                                stop=True,
                                tile_position=(po, 0),
                            )
                    emat = []
                    for par in range(2):
                        e = epool.tile([N, 4 * N], BF16, tag="e")
                        nc.scalar.activation(e[:], psS[par][0:N, :], AF.Exp)
                        emat.append(e)
                    # AV with ones-row: rows 0..63 unnormalized out, row 64 denom
                    psA = []
                    for par in range(2):
                        ps = ps_av.tile([D + 1, 4 * N], F32, tag="psav")
                        for hh in range(4):
                            h = 2 * hh + par
                            nc.tensor.matmul(
                                ps[:, hh * N:(hh + 1) * N],
                                lhsT=v_aug[j][0:N, h * (D + 1):(h + 1) * (D + 1)],
                                rhs=emat[par][:, hh * N:(hh + 1) * N],
                                start=True,
                                stop=True,
                            )
                        psA.append(ps)
                    # ot free layout is (b01, par, hh, n); channel h = 2*hh+par
                    # one copy on DVE, one on ACT to balance engine load
                    joff = (j % 2) * 2 * 4 * N
                    nc.vector.tensor_copy(
                        ot[:, joff:joff + 4 * N], psA[0][:]
                    )
                    nc.scalar.activation(
                        ot[:, joff + 4 * N:joff + 2 * 4 * N], psA[1][:], AF.Identity
                    )
                    if j % 2 == 1:
                        nc.sync.dma_start(out=outd[b // 2], in_=ot[:])

            # software pipeline: attention for chunk c-1 is emitted before
            # GEMM for chunk c so PE never stalls on ACT/DVE epilogues.
            # x loads are emitted one chunk ahead of their GEMM so the DMA
            # queues run a full chunk ahead of the consumers.
            xts = {0: xt0}
            for c in range(nchunks + 1):
                if c + 1 < nchunks:
                    xts[c + 1] = load_x(c + 1)
                if c > 0:
                    attention(state)
                if c < nchunks:
                    state = gemm(c, xt=xts.pop(c))

    if not nc.is_finalized():
        nc.finalize()
    return nc


_CACHE = {}


def _get_nc(n_b):
    if n_b not in _CACHE:
        _CACHE[n_b] = build_kernel(n_b)
    return _CACHE[n_b]


def _prep_inputs(x, qkv_w, qkv_b, rel_h, rel_w):
    # per-core channel-major x: [NCORES][DM, B_CORE*N] in fp16
    x = np.asarray(x, dtype=np.float32).reshape(B, DM, N)
    x = np.ascontiguousarray(
        x.reshape(NCORES, B_CORE, DM, N).transpose(0, 2, 1, 3)
    ).reshape(NCORES, DM, B_CORE * N).astype(np.float16)
    qkv_w = np.asarray(qkv_w, dtype=np.float32)
    qkv_b = np.asarray(qkv_b, dtype=np.float32)
    wt = np.ascontiguousarray(qkv_w.T).astype(np.float16)               # [512, 1536]
    bq = np.ascontiguousarray(qkv_b[0:DM].reshape(DM, 1))
    rel = (np.asarray(rel_h, np.float32) + np.asarray(rel_w, np.float32))
    rp = np.ascontiguousarray(rel.reshape(DM, N) + qkv_b[DM:2 * DM].reshape(DM, 1))
    bv = np.ascontiguousarray(qkv_b[2 * DM:3 * DM])
    return x, wt, bq, rp, bv


def kernel(x, qkv_w, qkv_b, rel_h, rel_w, _trace=False):
    xs, wt, bq, rp, bv = _prep_inputs(x, qkv_w, qkv_b, rel_h, rel_w)
    nc = _get_nc(B_CORE)
    in_maps = [
        {"x": xs[i], "wt": wt, "bq": bq, "rp": rp}
        for i in range(NCORES)
    ]
    res = run_bass_kernel_spmd(
        nc, in_maps, core_ids=list(range(NCORES)), trace=_trace
    )
    # decode device layout [pair, d|denom, b01, par, hh, n] -> [B, DM, N];
    # row D is the softmax denominator (normalize here during unshard), and
    # the v bias (linear in the attention average) is added after the divide
    out = np.stack(
        [np.asarray(r["out"]).astype(np.float32) for r in res.results], axis=0
    )
    out = out.reshape(NCORES, B_CORE // 2, D + 1, 2, 2, 4, N)
    out = out[:, :, 0:D] / out[:, :, D:D + 1]
    # axes now [core, pair, d, b01, par, hh, n]; channel h = 2*hh + par
    out = out + bv.reshape(4, 2, D).transpose(2, 1, 0)[None, None, :, None, :, :, None]
    out = out.transpose(0, 1, 3, 5, 4, 2, 6)  # core, pair, b01, hh, par, d, n
    out = out.reshape(B, DM, N)
    if _trace:
        kernel.last_results = res
    return np.ascontiguousarray(out.reshape(B, DM, 9, 9))


# revision 35
# speedup vs baseline: 1.0147x; 1.0006x over previous
"""Trainium2 Bass kernel for batched 9x9-token MHSA with decomposed relative
position bias (1x1-conv QKV projection).

Strategy: pure data parallel over batch (B=1024 -> 128 per core x 8 cores).
All matmuls run on 16-bit operands (fp16 GEMM/scores, bf16 AV) — fp32/fp32r
matmuls measure ~2.5-3.5 cycles/column on TRN2 HW while 16-bit runs at 1.

Per core:
  - QK projection GEMM channel-major (out [o, (b,n)]), fp16 operands,
    N=324 moving columns. Relative-position table R = rel_h+rel_w (+ k bias)
    is folded into K during the PSUM->SBUF epilogue, so scores = Q.(K+R).
  - V projection GEMM token-major over 128-row windows that span batch
    boundaries (full PE column utilization), fp16, N=512 moving; batch-
    aligned v tiles assembled via contiguous SBUF->SBUF shift DMAs.
  - Scores computed transposed: S^T[m,n] = sum_d k'[d,m] q[d,n] via
    matmul(lhsT=k', rhs=q) in fp16. Softmax runs over partitions (m): no max
    subtraction (logits bounded by ~33, exp<=1.4e14, safe in fp32);
    denominator obtained by appending a ones-row to V so the AV matmul
    emits unnormalized output rows 0..63 and the denominator in row 64.
  - exp on ScalarE (fp32 PSUM -> bf16 SBUF), AV matmul in bf16.
  - Output stored bf16 (halves write traffic); the final normalize division
    happens on the host during unsharding.

Self-contained: hardcodes B=1024, DM=512, H=8, D=64, N=81, 8 cores.
"""

import os
import sys

import numpy as np

for _p in ("/opt/trn_rl_repo", "/root/.axon_site/_ro/trn_rl_repo"):
    if os.path.isdir(_p) and _p not in sys.path:
        sys.path.insert(0, _p)

import concourse.bass as bass
import concourse.tile as tile
from concourse import bacc
from concourse import mybir
from concourse.alu_op_type import AluOpType
from concourse.bass_utils import run_bass_kernel_spmd

F32 = mybir.dt.float32
F16 = mybir.dt.float16
BF16 = mybir.dt.bfloat16
AF = mybir.ActivationFunctionType

B, DM, H, D, N = 1024, 512, 8, 64, 81
NCORES = 8
B_CORE = B // NCORES  # 128
NB = 4                # batches per chunk
NCOLS = NB * N        # 324 GEMM moving columns per chunk


def build_kernel(n_b=B_CORE):
    assert n_b % NB == 0
    nchunks = n_b // NB

    nc = bacc.Bacc()
    # x pre-transposed on host to channel-major [DM, n_b*N] so every DMA row
    # is a contiguous run; fp16 halves the HBM read traffic.
    xd = nc.dram_tensor("x", [DM, n_b * N], F16, kind="ExternalInput")
    wtd = nc.dram_tensor("wt", [DM, 3 * DM], F16, kind="ExternalInput")  # W^T
    bqd = nc.dram_tensor("bq", [DM, 1], F32, kind="ExternalInput")       # q bias
    rpd = nc.dram_tensor("rp", [DM, N], F32, kind="ExternalInput")       # rel_h+rel_w+bk
    # out in device-native layout [pair][d+denom][b par hh n]; row D holds the
    # softmax denominator — the final normalize division happens on the host
    # during unsharding. One fully-contiguous store per batch pair.
    outd = nc.dram_tensor(
        "out", [n_b // 2, D + 1, 2 * 2 * 4 * N], BF16, kind="ExternalOutput"
    )

    with tile.TileContext(nc) as tc:
        with (
            tc.tile_pool(name="const", bufs=1) as cpool,
            tc.tile_pool(name="xin", bufs=3) as xpool,
            tc.tile_pool(name="qk", bufs=2) as qkpool,
            tc.tile_pool(name="vaug", bufs=8) as vpool,
            tc.tile_pool(name="vstage", bufs=6) as svpool,
            tc.tile_pool(name="emat", bufs=4) as epool,
            tc.tile_pool(name="outs", bufs=3) as opool,
            tc.tile_pool(name="ps_qk", bufs=3, space="PSUM") as ps_qk,
            tc.tile_pool(name="ps_v", bufs=1, space="PSUM") as ps_v,
            tc.tile_pool(name="ps_s", bufs=2, space="PSUM") as ps_s,
            tc.tile_pool(name="ps_av", bufs=2, space="PSUM") as ps_av,
        ):
            # ---- first x chunk + constants; ordered so the first QK matmul
            # group unblocks as early as possible: x(0) and the q-section of
            # the weights land first, then k/v sections and the small tables.
            def load_x(c):
                b0 = c * NB
                xt = []
                for kc in range(4):
                    t = xpool.tile([128, NB, N], F16, tag=f"x{kc}")
                    nc.sync.dma_start(
                        out=t[:].rearrange("p b n -> p (b n)"),
                        in_=xd[kc * 128:(kc + 1) * 128, b0 * N:(b0 + NB) * N],
                    )
                    xt.append(t)
                return xt

            # warm-up: the PE clock gate (HAM) starts at half rate and needs
            # ~3.4us of sustained activity to release. The first ~12us of the
            # kernel are DMA wait anyway, so burn them on dummy matmuls over
            # never-initialized scratch (results discarded) to enter the real
            # GEMM at full clock.
            scratch = cpool.tile([128, NCOLS], F16, tag="scratch")
            nc.vector.memset(scratch[:], 0.0)
            wps = ps_qk.tile([128, NCOLS], F32, tag="psqk", name="warm")
            for _ in range(60):
                nc.tensor.matmul(
                    wps[:], lhsT=scratch[:, 0:128], rhs=scratch[:],
                    start=True, stop=True,
                )

            xt0 = load_x(0)
            wt = [
                cpool.tile([128, 3 * DM], F16, tag=f"wt{kc}", name=f"wt{kc}")
                for kc in range(4)
            ]
            for sec in range(3):  # q, k, v weight column sections
                for kc in range(4):
                    nc.sync.dma_start(
                        out=wt[kc][:, sec * DM:(sec + 1) * DM],
                        in_=wtd[kc * 128:(kc + 1) * 128, sec * DM:(sec + 1) * DM],
                    )
                if sec == 0:
                    rp = []
                    for mo in range(4):
                        t = cpool.tile([128, N], F32, tag=f"rp{mo}")
                        nc.sync.dma_start(
                            out=t[:], in_=rpd[mo * 128:(mo + 1) * 128, :]
                        )
                        rp.append(t)
                elif sec == 1:
                    bq = []
                    for mo in range(4):
                        t = cpool.tile([128, 1], F32, tag=f"bq{mo}")
                        nc.sync.dma_start(
                            out=t[:], in_=bqd[mo * 128:(mo + 1) * 128, :]
                        )
                        bq.append(t)
            state = {}  # carries one chunk's tiles to the next iteration

            # V window segment map: chunk token-row g = 81*j + t is computed
            # in window w at row g-WB[w] and must land in va[j] row t.
            # Uneven boundaries [0,128,243,324) make batches 0 AND 3 start
            # window-aligned so both alias their window tile (no shift DMA).
            # (w, src_row0, src_row1, batch j, dst_row0)
            WB = [0, 128, 243, NB * N]
            vsegs = []
            for j in range(NB):
                t = 0
                while t < N:
                    g = j * N + t
                    w = max(i for i in range(3) if WB[i] <= g)
                    seg = min(N - t, WB[w + 1] - g)
                    vsegs.append((w, g - WB[w], g - WB[w] + seg, j, t))
                    t += seg
            aliased = {
                j: segs[0][0]
                for j in range(NB)
                if len(segs := [s for s in vsegs if s[3] == j]) == 1
                and segs[0][1] == 0
            }

            def gemm(c, xt=None):
                b0 = c * NB
                if xt is None:
                    xt = load_x(c)

                # q,k channel-major GEMM: out[o, (b,n)] for o in 0..1024
                q_sb, k_sb = [], []
                for mo in range(8):
                    ps = ps_qk.tile([128, NCOLS], F32, tag="psqk")
                    for kc in range(4):
                        nc.tensor.matmul(
                            ps[:],
                            lhsT=wt[kc][:, mo * 128:(mo + 1) * 128],
                            rhs=xt[kc][:].rearrange("p b n -> p (b n)"),
                            start=(kc == 0),
                            stop=(kc == 3),
                        )
                    if mo < 4:  # q: add bias on ScalarE while copying out
                        t = qkpool.tile([128, NCOLS], F16, tag=f"q{mo}")
                        nc.scalar.activation(t[:], ps[:], AF.Identity, bias=bq[mo][:])
                        q_sb.append(t)
                    else:  # k: add (rel bias + k bias), broadcast over batch
                        # tile padded to NCOLS+47 so every scores lhsT slice
                        # can be 128 columns wide -> LDWEIGHTS qualifies for
                        # fast weight load. Pad contents are never-read junk
                        # (scores output rows 81..127 are ignored).
                        t = qkpool.tile([128, NCOLS + 47], F16, tag=f"k{mo - 4}")
                        nc.vector.tensor_tensor(
                            t[:, 0:NCOLS].rearrange("p (b n) -> p b n", b=NB),
                            ps[:].rearrange("p (b n) -> p b n", b=NB),
                            rp[mo - 4][:].unsqueeze(1).broadcast_to([128, NB, N]),
                            AluOpType.add,
                        )
                        k_sb.append(t)

                # v token-major GEMM in 128-row windows spanning batch
                # boundaries (full PE column utilization: 3 windows/chunk
                # instead of 4 81-row batches). No bias — host adds it after
                # the final normalize. Misaligned rows reach their va tile by
                # SBUF->SBUF shift DMA via a bf16 staging tile.
                # staging is written in the augmented (h, d|1) layout by the
                # DVE cast (strided dst is free: PSUM src caps DVE at 1x
                # anyway), so every shift DMA is fully contiguous rows.
                sv = []
                for w in range(3):
                    r0, r1 = WB[w], WB[w + 1]
                    rows = r1 - r0
                    ps = ps_v.tile([128, DM], F32, tag="psv")
                    for kc in range(4):
                        nc.tensor.matmul(
                            ps[0:rows, :],
                            lhsT=xt[kc][:].rearrange("p b n -> p (b n)")[:, r0:r1],
                            rhs=wt[kc][:, 2 * DM:3 * DM],
                            start=(kc == 0),
                            stop=(kc == 3),
                        )
                    t = svpool.tile([128, H * (D + 1)], BF16, tag="sv", name=f"sv{w}")
                    nc.vector.tensor_copy(
                        t[0:rows].rearrange("p (h e) -> p h e", h=H)[:, :, 0:D],
                        ps[0:rows, :].rearrange("p (h d) -> p h d", h=H),
                    )
                    nc.vector.memset(
                        t[0:rows].rearrange("p (h e) -> p h e", h=H)[:, :, D:D + 1],
                        1.0,
                    )
                    sv.append(t)
                # window-aligned batches alias their window tile directly;
                # the rest are assembled by contiguous shift DMAs.
                v_aug = []
                for j in range(NB):
                    if j in aliased:
                        v_aug.append(sv[aliased[j]])
                    else:
                        va = vpool.tile(
                            [N, H * (D + 1)], BF16, tag="vaug", name=f"va{j}"
                        )
                        v_aug.append(va)
                for w, s0, s1, j, t0 in vsegs:
                    if j in aliased:
                        continue
                    nc.sync.dma_start(
                        out=v_aug[j][t0:t0 + (s1 - s0), :],
                        in_=sv[w][s0:s1, :],
                    )
                return {"q": q_sb, "k": k_sb, "v": v_aug, "b0": b0}

            def attention(st):
                q_sb, k_sb, v_aug, b0 = st["q"], st["k"], st["v"], st["b0"]
                ot = None
                for j in range(NB):
                    b = b0 + j
                    js = slice(j * N, (j + 1) * N)
                    if j % 2 == 0:  # one output tile per batch pair
                        ot = opool.tile([D + 1, 2 * 2 * 4 * N], BF16, tag="ot")
                    # scores transposed: S^T = k'.T-contracted over d.
                    # Grouped by head parity: tile `par` holds heads 2*hh+par,
                    # so every matmul into one PSUM tile has the same lhsT
                    # base partition (mixing 0/64 in one fp32 group breaks HW).
                    # parities interleaved: consecutive matmuls use disjoint
                    # PE row strips (0-63 vs 64-127) and different PSUM banks,
                    # so the PE can overlap them
                    psS = [
                        ps_s.tile([128, 4 * N], F32, tag="pss", name=f"pss{j}_{p}")
                        for p in range(2)
                    ]
                    for hh in range(4):
                        for par in range(2):
                            po = par * 64
                            # 128-wide lhsT slice (81 real tokens + padding)
                            # enables FWL on the weight load; output rows
                            # 81..127 are junk and never read.
                            nc.tensor.matmul(
                                psS[par][:, hh * N:(hh + 1) * N],
                                lhsT=k_sb[hh][po:po + 64, j * N:j * N + 128],
                                rhs=q_sb[hh][po:po + 64, js],
                                start=True,
                                stop=True,
                                tile_position=(po, 0),
                            )
                    emat = []
                    for par in range(2):
                        e = epool.tile([N, 4 * N], BF16, tag="e")
                        nc.scalar.activation(e[:], psS[par][0:N, :], AF.Exp)
                        emat.append(e)
                    # AV with ones-row: rows 0..63 unnormalized out, row 64 denom
                    psA = []
                    for par in range(2):
                        ps = ps_av.tile([D + 1, 4 * N], F32, tag="psav")
                        for hh in range(4):
                            h = 2 * hh + par
                            nc.tensor.matmul(
                                ps[:, hh * N:(hh + 1) * N],
                                lhsT=v_aug[j][0:N, h * (D + 1):(h + 1) * (D + 1)],
                                rhs=emat[par][:, hh * N:(hh + 1) * N],
                                start=True,
                                stop=True,
                            )
                        psA.append(ps)
                    # ot free layout is (b01, par, hh, n); channel h = 2*hh+par
                    # one copy on DVE, one on ACT to balance engine load
                    joff = (j % 2) * 2 * 4 * N
                    nc.vector.tensor_copy(
                        ot[:, joff:joff + 4 * N], psA[0][:]
                    )
                    nc.scalar.activation(
                        ot[:, joff + 4 * N:joff + 2 * 4 * N], psA[1][:], AF.Identity
                    )
                    if j % 2 == 1:
                        nc.sync.dma_start(out=outd[b // 2], in_=ot[:])

            # software pipeline: attention for chunk c-1 is emitted before
            # GEMM for chunk c so PE never stalls on ACT/DVE epilogues
            for c in range(nchunks + 1):
                if c > 0:
                    attention(state)
                if c < nchunks:
                    state = gemm(c, xt=xt0 if c == 0 else None)

    if not nc.is_finalized():
        nc.finalize()
    return nc


_CACHE = {}


def _get_nc(n_b):
    if n_b not in _CACHE:
        _CACHE[n_b] = build_kernel(n_b)
    return _CACHE[n_b]


def _prep_inputs(x, qkv_w, qkv_b, rel_h, rel_w):
    # per-core channel-major x: [NCORES][DM, B_CORE*N] in fp16
    x = np.asarray(x, dtype=np.float32).reshape(B, DM, N)
    x = np.ascontiguousarray(
        x.reshape(NCORES, B_CORE, DM, N).transpose(0, 2, 1, 3)
    ).reshape(NCORES, DM, B_CORE * N).astype(np.float16)
    qkv_w = np.asarray(qkv_w, dtype=np.float32)
    qkv_b = np.asarray(qkv_b, dtype=np.float32)
    wt = np.ascontiguousarray(qkv_w.T).astype(np.float16)               # [512, 1536]
    bq = np.ascontiguousarray(qkv_b[0:DM].reshape(DM, 1))
    rel = (np.asarray(rel_h, np.float32) + np.asarray(rel_w, np.float32))
    rp = np.ascontiguousarray(rel.reshape(DM, N) + qkv_b[DM:2 * DM].reshape(DM, 1))
    bv = np.ascontiguousarray(qkv_b[2 * DM:3 * DM])
    return x, wt, bq, rp, bv


def kernel(x, qkv_w, qkv_b, rel_h, rel_w, _trace=False):
    xs, wt, bq, rp, bv = _prep_inputs(x, qkv_w, qkv_b, rel_h, rel_w)
    nc = _get_nc(B_CORE)
    in_maps = [
        {"x": xs[i], "wt": wt, "bq": bq, "rp": rp}
        for i in range(NCORES)
    ]
    res = run_bass_kernel_spmd(
        nc, in_maps, core_ids=list(range(NCORES)), trace=_trace
    )
    # decode device layout [pair, d|denom, b01, par, hh, n] -> [B, DM, N];
    # row D is the softmax denominator (normalize here during unshard), and
    # the v bias (linear in the attention average) is added after the divide
    out = np.stack(
        [np.asarray(r["out"]).astype(np.float32) for r in res.results], axis=0
    )
    out = out.reshape(NCORES, B_CORE // 2, D + 1, 2, 2, 4, N)
    out = out[:, :, 0:D] / out[:, :, D:D + 1]
    # axes now [core, pair, d, b01, par, hh, n]; channel h = 2*hh + par
    out = out + bv.reshape(4, 2, D).transpose(2, 1, 0)[None, None, :, None, :, :, None]
    out = out.transpose(0, 1, 3, 5, 4, 2, 6)  # core, pair, b01, hh, par, d, n
    out = out.reshape(B, DM, N)
    if _trace:
        kernel.last_results = res
    return np.ascontiguousarray(out.reshape(B, DM, 9, 9))
